# revision 1
# baseline (speedup 1.0000x reference)
"""BlockLinear kernel for Trainium2 (8 NeuronCores, SPMD).

y[b, g*512+o] = sum_i x[b, g*512+i] * W[g, o, i] + bias[g, o]

Sharding: one block g per core (expert parallelism). Each core computes
y_g = x_g @ W_g^T + b_g with x_g = x[:, g*512:(g+1)*512].

Per-core device kernel (fp16 in/compute, fp16 out, rel err ~3e-4,
~130us HW time; PE-bound: 512 matmuls x 216.8ns = 111us of streaming
at the 1 col/cycle fp16 limit, plus ~12us of fixed NEFF preamble +
DMA-visibility latency at the head and ~6us of output-receipt +
drain/epilogue at the tail):
  - All device-side DMAs are fully contiguous per partition: the host
    pre-arranges x into the per-group SBUF tile layout ([p, k, b] per
    group, concatenated) and the device writes y in its tile-native
    layout ([p, subtile, o] per group) which the host un-permutes.
    This keeps HWDGE descriptor counts minimal (128 per transfer).
  - x-group DMAs ride the sync HWDGE queue; weights (two contiguous
    halves) ride the scalar queue; y-out DMAs ride sync behind x so
    input fetch is never delayed by an output DMA waiting on a DVE
    semaphore (FIFO order puts each x ahead of same-group y).
  - batch processed in groups (ramp 256..1536 while the DMA pipeline
    fills behind 5 fp32 warmup matmuls, 3072-row body, descending
    tail): per 128-row subtile 4 accumulating fp16 matmuls into one of
    8 cycling PSUM banks, DVE adds bias while copying PSUM->SBUF
    (casting to fp16), group result DMA'd out contiguously. The final
    subtile drains in two column halves whose output DMAs issue on
    both HWDGE queues in parallel (512B/partition descriptors stay at
    DMA line rate; the last DMA's ~2.5us completion receipt dominates
    the tail).

Rejected alternatives (measured on this hardware): fp8 DoubleRow
matmuls run at the same 216ns per instruction as fp16 (2x FLOPs via
K=256 per pass), but e4m3 quantization costs 2.7-3.8e-2 relative
error against a 2e-2 gate, and the scaled hi/lo compensation scheme
that passes the gate (1.2e-3) needs 3 passes -> 1.5x the fp16 MM
count. float32r matches fp16 MM speed but doubles DMA bytes.
"""

import numpy as np

import concourse.bass as bass
import concourse.mybir as mybir
import concourse.tile as tile
from concourse import bacc
from concourse.bass_utils import run_bass_kernel_spmd
from concourse.vector_clock import ScopedClock

F32 = mybir.dt.float32

NB, BIN, BOUT = 8, 512, 512
BATCH = 16384
NCORES = 8
P = 128
KT = BIN // P  # 4 k-tiles per block

SCHEME = "f16"  # "f16" | "f16_f32out" | "f32r" | "f32"

_patched = False


def _patch_tile_drain():
    """Walrus in this container accepts only one sync-wait per InstDrain;
    split the tile-exit drain's waits across one drain instruction each."""
    global _patched
    if _patched:
        return
    _patched = True

    def _drain_and_barrier(self, tick_clock, wait_clock):
        nc = self.nc
        drain_inst = nc.sync.drain()
        wait_clock.add_sem_waits(
            drain_inst.ins, ScopedClock({None: tick_clock.global_clock})
        )
        si = drain_inst.ins.sync_info
        if si is not None and len(si.on_wait) > 1:
            waits = list(si.on_wait)
            updates = list(si.on_update)
            drain_inst.ins.sync_info = mybir.SyncInfo(
                on_wait=[waits[0]], on_update=updates
            )
            for w in waits[1:]:
                extra = nc.sync.drain()
                extra.ins.sync_info = mybir.SyncInfo(on_wait=[w], on_update=[])
        nc.all_engine_barrier()
        popped = nc._tile_sem_poison_stack.pop()
        assert popped is self._sem_poison
        # Skip Tile's exit-time sem clear + second barrier: walrus's
        # end-of-NEFF epilogue unconditionally zeroes every semaphore on
        # every engine, and nothing runs between the barrier above and
        # that epilogue. (Verified: repeated executions stay correct.)
        sems = list(self.sems.allocated().values())
        sem_nums = [s.num if hasattr(s, "num") else s for s in sems]
        nc._state.prepend_free_semaphores(sem_nums)
        for poison_set in nc._tile_sem_poison_stack:
            poison_set.update(sem_nums)

    tile.TileContext._drain_and_barrier = _drain_and_barrier


_nc_cache = {}


def _scheme_dtypes(scheme):
    if scheme in ("f16", "f16_f32out"):
        in_dt = mybir.dt.float16
        out_dt = mybir.dt.float16 if scheme == "f16" else F32
        np_in = np.float16
    elif scheme == "f32r":
        in_dt, out_dt, np_in = mybir.dt.float32r, F32, np.float32
    elif scheme == "f32":
        in_dt, out_dt, np_in = F32, F32, np.float32
    else:
        raise ValueError(scheme)
    return in_dt, out_dt, np_in


def _groups(group):
    """Batch-row group sizes: geometric ramp at the start (matmuls start on
    the first small group while DMA builds runway) and a descending tail
    (the final output DMAs are small and the ring is drained by the end)."""
    head = [256, 512, 1024, 1536]
    # descending tail; the second-to-last groups are small so their output
    # flushes clear the ring before the final group's output DMAs issue
    tail = [1536, 512, 512, 256]
    body = BATCH - sum(head) - sum(tail)
    sizes = head + [group] * (body // group)
    rem = body % group
    if rem:
        sizes.append(rem)
    sizes += tail
    assert sum(sizes) == BATCH and all(s % P == 0 for s in sizes), sizes
    return sizes


def _build(scheme=SCHEME, group=3072):
    key = (scheme, group)
    if key in _nc_cache:
        return _nc_cache[key]
    _patch_tile_drain()
    in_dt, out_dt, _ = _scheme_dtypes(scheme)

    nc = bacc.Bacc(None, target_bir_lowering=False)
    # x pre-arranged by host into per-group tile layout, fully contiguous
    xP = nc.dram_tensor("xP", [P, KT * BATCH], in_dt, kind="ExternalInput")
    # weights pre-arranged to [p, k, o] (contiguous load)
    wP = nc.dram_tensor("wP", [P, KT * BOUT], in_dt, kind="ExternalInput")
    bias = nc.dram_tensor("bias", [P, BOUT], F32, kind="ExternalInput")
    # y in device tile layout [p, subtile, o]; host un-permutes
    yP = nc.dram_tensor("yP", [P, (BATCH // P) * BOUT], out_dt, kind="ExternalOutput")

    with tile.TileContext(nc) as tc:
        with (
            tc.tile_pool(name="const", bufs=1) as const,
            tc.tile_pool(name="xp", bufs=4) as xp,
            tc.tile_pool(name="yp", bufs=4) as yp,
            tc.tile_pool(name="ps", bufs=8, space="PSUM") as psp,
        ):
            # PE warmup: dummy fp32 matmuls with no DMA dependency keep the
            # PE busy (warming the HAM clock-gate) while the first x/w
            # transfers are in flight.
            scratch = const.tile([P, 384], F32)
            nc.vector.memset(scratch[:], 0.0)
            warm_ps = psp.tile([P, BOUT], F32, tag="ps")
            for _ in range(5):
                nc.tensor.matmul(
                    warm_ps[:, :256],
                    scratch[:, :P],
                    scratch[:, P:],
                    start=True,
                    stop=True,
                )

            # Weights in two contiguous half DMAs on the scalar queue: the
            # k=0/1 half (needed by the first matmuls) becomes consumer-
            # visible earlier than the k=2/3 half; x groups own the sync
            # queue so the first x group is never queued behind weights.
            wt = const.tile([P, KT, BOUT], in_dt)
            wflat = wt[:].rearrange("p k o -> p (k o)")
            half = KT * BOUT // 2
            nc.scalar.dma_start(wflat[:, :half], wP[:, :half])
            nc.scalar.dma_start(wflat[:, half:], wP[:, half:])
            bt = const.tile([P, BOUT], F32)

            sizes = _groups(group)
            last = len(sizes) - 1
            row = 0
            for mg, gsz in enumerate(sizes):
                nsub = gsz // P
                xt = xp.tile([P, KT, gsz], in_dt, tag="xt")
                nc.sync.dma_start(
                    xt[:].rearrange("p k b -> p (k b)"),
                    xP[:, KT * row : KT * (row + gsz)],
                )
                if mg == 0:
                    # bias rides sync after the first x group; first needed
                    # by the first DVE add, well after the first matmul
                    nc.sync.dma_start(bt[:], bias[:])
                yt = yp.tile([P, nsub, BOUT], out_dt, tag="yt")
                for ms in range(nsub):
                    ps = psp.tile([P, BOUT], F32, tag="ps")
                    for k in range(KT):
                        nc.tensor.matmul(
                            ps[:],
                            xt[:, k, ms * P : (ms + 1) * P],
                            wt[:, k, :],
                            start=(k == 0),
                            stop=(k == KT - 1),
                        )
                    if mg == last and ms == nsub - 1:
                        # drain the final subtile in two column halves so the
                        # first output DMA issues while the DVE adds bias to
                        # the second half (512B/partition descriptors keep
                        # both final DMAs at DMA line rate)
                        for q in range(2):
                            c0, c1 = q * 256, (q + 1) * 256
                            nc.vector.tensor_add(
                                out=yt[:, ms, c0:c1], in0=ps[:, c0:c1], in1=bt[:, c0:c1]
                            )
                    else:
                        nc.vector.tensor_add(out=yt[:, ms, :], in0=ps[:], in1=bt[:])
                yoff = (row // P) * BOUT
                ydst = yP[:, yoff : yoff + nsub * BOUT]
                if mg == last:
                    base = (nsub - 1) * BOUT
                    if nsub > 1:
                        nc.sync.dma_start(ydst[:, :base], yt[:, : nsub - 1, :])
                    nc.scalar.dma_start(
                        ydst[:, base : base + 256], yt[:, nsub - 1, :256]
                    )
                    nc.sync.dma_start(
                        ydst[:, base + 256 : base + 512], yt[:, nsub - 1, 256:]
                    )
                else:
                    nc.sync.dma_start(ydst, yt[:])
                row += gsz
    nc.compile()
    _nc_cache[key] = nc
    return nc


LAST_RESULT = None


def kernel(x, W, b, trace=False, scheme=SCHEME, group=3072, trace_kwargs=None):
    global LAST_RESULT
    x = np.asarray(x, dtype=np.float32)
    W = np.asarray(W, dtype=np.float32)
    b = np.asarray(b, dtype=np.float32)

    _, _, np_in = _scheme_dtypes(scheme)
    nc = _build(scheme, group)
    sizes = _groups(group)

    in_maps = []
    for g in range(NCORES):
        # x_g^T as [p, k, b] (k-tile-major per partition), then sliced into
        # the fixed group schedule and flattened per group
        xk = np.ascontiguousarray(
            x[:, g * BIN : (g + 1) * BIN].T.astype(np_in).reshape(KT, P, BATCH).transpose(1, 0, 2)
        )  # [P, KT, BATCH]
        blocks = []
        r = 0
        for gsz in sizes:
            blocks.append(xk[:, :, r : r + gsz].reshape(P, KT * gsz))
            r += gsz
        xP_g = np.ascontiguousarray(np.concatenate(blocks, axis=1))
        wP_g = np.ascontiguousarray(
            W[g].T.astype(np_in).reshape(KT, P, BOUT).transpose(1, 0, 2).reshape(P, KT * BOUT)
        )
        bias_g = np.ascontiguousarray(np.broadcast_to(b[g][None, :], (P, BOUT)))
        in_maps.append({"xP": xP_g, "wP": wP_g, "bias": bias_g})

    kwargs = dict(trace_kwargs or {})
    res = run_bass_kernel_spmd(nc, in_maps, list(range(NCORES)), trace=trace, **kwargs)
    LAST_RESULT = res

    out = np.empty((BATCH, NB * BOUT), dtype=np.float32)
    for g in range(NCORES):
        # yP [P, (BATCH/P)*BOUT] -> y rows: subtile s holds rows s*P..s*P+127
        yp_g = res.results[g]["yP"].reshape(P, BATCH // P, BOUT).transpose(1, 0, 2)
        out[:, g * BOUT : (g + 1) * BOUT] = yp_g.reshape(BATCH, BOUT).astype(np.float32)
    return out

